# revision 6
# baseline (speedup 1.0000x reference)
"""Cheb-GCN (graph conv + batchnorm) Trainium2 kernel, 8-core data parallel.

out[b,o,m,t] = BN(relu( sum_{k,c,n} Theta[k,c,o] * relu(x[b,c,n,t]) * Tk[k,n,m] ))
Tk[k] = softmax(alpha[k])[0]*cheb[k] + softmax(alpha[k])[1]*adp
adp   = row_softmax(mask0(relu(nv1@nv2)))

Sharding: batch (64) split 8 ways; adjacency/Theta replicated; BN stats
all-reduced on device across the 8 cores.
"""
import sys
import numpy as np

if '/opt/trn_rl_repo' not in sys.path:
    sys.path.insert(0, '/opt/trn_rl_repo')

B, C, O, N, T, K, D = 64, 64, 64, 512, 12, 3, 10
EPS = 1e-5
NCORES = 8
BLOC = B // NCORES          # 8 batches per core
NPAIRS = BLOC // 2          # 4 (slot pairs: b = slot*NPAIRS + pair)
NCHUNKS = N // 128          # 4
NKCH = K * NCHUNKS          # 12 contraction chunks of 128 for stage 2

_CACHE = {}


def _build(n_cores=NCORES):
    import concourse.bacc as bacc
    from concourse import mybir, tile

    dt = mybir.dt
    f32, f32r = dt.float32, dt.float32r
    AF = mybir.ActivationFunctionType
    ALU = mybir.AluOpType
    AX = mybir.AxisListType
    cnt = float(BLOC * n_cores * N * T)   # batchnorm reduction count per channel

    nc = bacc.Bacc(trn_type="TRN2", num_devices=n_cores)

    x_ext = nc.dram_tensor("x", [BLOC, C, N, T], f32, kind="ExternalInput")
    cheb_ext = nc.dram_tensor("cheb", [K, N, N], f32, kind="ExternalInput")
    nv1_ext = nc.dram_tensor("nodevec1", [N, D], f32, kind="ExternalInput")
    nv2_ext = nc.dram_tensor("nodevec2", [D, N], f32, kind="ExternalInput")
    alpha_ext = nc.dram_tensor("alpha", [K, 2], f32, kind="ExternalInput")
    theta_ext = nc.dram_tensor("Theta", [K, C, O], f32, kind="ExternalInput")
    out_ext = nc.dram_tensor("out", [BLOC, O, N, T], f32, kind="ExternalOutput")

    with tile.TileContext(nc) as tc:
        with tc.tile_pool(name="const", bufs=1) as cp, \
             tc.tile_pool(name="pp", bufs=2, space="PSUM") as pp, \
             tc.tile_pool(name="dram", bufs=1, space="DRAM") as dram:

            # ---- persistent tiles ----
            # G[:, k*4+nchunk, :] = Tk[k][nchunk*128:(nchunk+1)*128, :]
            G = cp.tile([128, NKCH, N], f32r)
            # block-diag Theta (fp32r): rows (slot,c), cols (slot, kpad4, o)
            th2 = cp.tile([128, 2, 4, O], f32r)
            # pre-BN output, rows (slot,o), cols (pair, m, t)
            out_sb = cp.tile([128, NPAIRS, N, T], f32)
            sums = cp.tile([128, NPAIRS * T], f32)
            ssq = cp.tile([128, NPAIRS * T], f32)
            ab_bc = cp.tile([128, 2 * K], f32)        # broadcast softmax(alpha)
            s_tile = cp.tile([128, NCHUNKS, K], f32)  # rinv * ab[k,1] per chunk-row

            # ================= prep phase (scratch pool, freed before main) ==
            with tc.tile_pool(name="prep", bufs=1) as prep:
                th2s = prep.tile([128, 2, 4, O], f32)
                adp = [prep.tile([128, N], f32, name=f"adp{i}")
                       for i in range(NCHUNKS)]
                rinv = [prep.tile([128, 1], f32, name=f"rinv{i}")
                        for i in range(NCHUNKS)]

                # ---- alpha softmax + broadcast ----
                al = prep.tile([K, 2], f32)
                nc.gpsimd.dma_start(out=al[:], in_=alpha_ext[:])
                amax = prep.tile([K, 1], f32)
                nc.vector.tensor_reduce(amax[:], al[:], AX.XYZW, ALU.max,
                                        negate=True)
                asum = prep.tile([K, 1], f32)
                nc.scalar.activation(al[:], al[:], AF.Exp, bias=amax[:],
                                     accum_out=asum[:])
                ainv = prep.tile([K, 1], f32)
                nc.vector.reciprocal(ainv[:], asum[:])
                nc.vector.tensor_scalar(al[:], al[:], ainv[:], None, ALU.mult)
                ab_flat = prep.tile([1, 2 * K], f32)
                nc.gpsimd.dma_start(out=ab_flat[:], in_=al[:])  # [3,2]->[1,6]
                ones = prep.tile([1, 128], f32)
                nc.vector.memset(ones[:], 1.0)
                ps_ab = pp.tile([128, 2 * K], f32, tag="ps_ab")
                nc.tensor.matmul(ps_ab[:], ones[:], ab_flat[:],
                                 start=True, stop=True)
                nc.scalar.copy(ab_bc[:], ps_ab[:])

                # ---- adaptive adjacency ----
                nv1t = prep.tile([D, N], f32)
                nc.gpsimd.dma_start(out=nv1t[:],
                                    in_=nv1_ext.rearrange("n d -> d n"))
                nv2s = prep.tile([D, N], f32)
                nc.gpsimd.dma_start(out=nv2s[:], in_=nv2_ext[:])
                for i in range(NCHUNKS):
                    ps = pp.tile([128, N], f32, tag="ps_adp")
                    nc.tensor.matmul(ps[:], nv1t[:, i * 128:(i + 1) * 128],
                                     nv2s[:], start=True, stop=True)
                    nc.scalar.activation(adp[i][:], ps[:], AF.Relu)
                    scr = prep.tile([128, N], f32, tag="adpscr", bufs=2)
                    # masked = adp + (adp==0)*(-1e10)
                    nc.vector.tensor_scalar(scr[:], adp[i][:], 0.0, None,
                                            ALU.is_equal)
                    nc.vector.scalar_tensor_tensor(adp[i][:], scr[:], -1e10,
                                                   adp[i][:], ALU.mult, ALU.add)
                    nmax = prep.tile([128, 1], f32, tag="nmax", bufs=2)
                    nc.vector.tensor_reduce(nmax[:], adp[i][:], AX.XYZW,
                                            ALU.max, negate=True)
                    rs = prep.tile([128, 1], f32, tag="rs", bufs=2)
                    nc.scalar.activation(adp[i][:], adp[i][:], AF.Exp,
                                         bias=nmax[:], accum_out=rs[:])
                    nc.vector.reciprocal(rinv[i][:], rs[:])
                    for k in range(K):
                        nc.vector.tensor_scalar(s_tile[:, i, k:k + 1],
                                                rinv[i][:],
                                                ab_bc[:, 2 * k + 1:2 * k + 2],
                                                None, ALU.mult)

                # ---- G = ab0*cheb + (exp_adp * s) ----
                for k in range(K):
                    for i in range(NCHUNKS):
                        cst = prep.tile([128, N], f32, tag="cheb_st", bufs=2)
                        nc.gpsimd.dma_start(
                            out=cst[:],
                            in_=cheb_ext[k, i * 128:(i + 1) * 128, :])
                        gv = G[:, k * NCHUNKS + i, :]
                        nc.vector.tensor_scalar(gv, cst[:],
                                                ab_bc[:, 2 * k:2 * k + 1],
                                                None, ALU.mult)
                        nc.vector.scalar_tensor_tensor(gv, adp[i][:],
                                                       s_tile[:, i, k:k + 1],
                                                       gv, ALU.mult, ALU.add)

                # ---- block-diag Theta ----
                nc.vector.memset(th2s[:], 0.0)
                for slot in range(2):
                    nc.gpsimd.dma_start(
                        out=th2s[slot * C:(slot + 1) * C, slot, 0:K, :],
                        in_=theta_ext.rearrange("k c o -> c k o"))
                nc.vector.tensor_copy(th2[:], th2s[:])

            # ================= main loop =================
            with tc.tile_pool(name="work", bufs=1) as wp:
                for pair in range(NPAIRS):
                    xp = wp.tile([128, N, T], f32r, tag="xp", bufs=2)
                    for i in range(NCHUNKS):
                        sl = slice(i * 128, (i + 1) * 128)
                        stg = wp.tile([128, 128, T], f32, tag="xstg", bufs=3)
                        nc.gpsimd.dma_start(out=stg[0:C], in_=x_ext[pair, :, sl, :])
                        nc.gpsimd.dma_start(out=stg[C:128],
                                            in_=x_ext[pair + NPAIRS, :, sl, :])
                        # relu(x), rounding to fp32r, alternate engines
                        if i % 2 == 0:
                            nc.vector.tensor_scalar(xp[:, sl, :], stg[:], 0.0,
                                                    None, ALU.max)
                        else:
                            nc.scalar.activation(xp[:, sl, :], stg[:], AF.Relu)

                    for t in range(T):
                        Y = wp.tile([128, K, NCHUNKS, 2, O], f32r, tag="Y",
                                    bufs=2)
                        for i in range(NCHUNKS):
                            psy = pp.tile([128, 2, 4, O], f32, tag="psy")
                            nc.tensor.matmul(
                                psy[:],
                                xp[:, i * 128:(i + 1) * 128, t],
                                th2[:], start=True, stop=True)
                            src = psy[:, :, 0:K, :].rearrange("p s k o -> p k s o")
                            if i % 2 == 0:
                                nc.vector.tensor_copy(Y[:, :, i, :, :], src)
                            else:
                                nc.scalar.copy(Y[:, :, i, :, :], src)
                        pso = pp.tile([128, N], f32, tag="pso")
                        for k in range(K):
                            for i in range(NCHUNKS):
                                c12 = k * NCHUNKS + i
                                nc.tensor.matmul(
                                    pso[:], Y[:, k, i, :, :], G[:, c12, :],
                                    start=(c12 == 0), stop=(c12 == NKCH - 1))
                        col = pair * T + t
                        osl = out_sb[:, pair, :, t]
                        nc.scalar.activation(osl, pso[:], AF.Relu,
                                             accum_out=sums[:, col:col + 1])
                        scr2 = wp.tile([128, N], f32, tag="sqscr", bufs=2)
                        nc.scalar.activation(scr2[:], osl, AF.Square,
                                             accum_out=ssq[:, col:col + 1])

                # ---- batchnorm stats + allreduce ----
                st2 = wp.tile([128, 2], f32)
                nc.vector.tensor_reduce(st2[:, 0:1], sums[:], AX.XYZW, ALU.add)
                nc.vector.tensor_reduce(st2[:, 1:2], ssq[:], AX.XYZW, ALU.add)
                hi = wp.tile([O, 2], f32)
                nc.gpsimd.dma_start(out=hi[:], in_=st2[O:128, :])
                cc_in = wp.tile([O, 2], f32)
                nc.vector.tensor_add(cc_in[:], st2[0:O, :], hi[:])

                cc_in_d = dram.tile([O, 2], f32)
                cc_out_d = dram.tile([O, 2], f32, addr_space="Shared")
                nc.gpsimd.dma_start(out=cc_in_d[:], in_=cc_in[:])
                nc.gpsimd.collective_compute(
                    "AllReduce", ALU.add,
                    replica_groups=[list(range(n_cores))],
                    ins=[cc_in_d.opt()], outs=[cc_out_d.opt()])
                gst = wp.tile([O, 2], f32)
                nc.gpsimd.dma_start(out=gst[:], in_=cc_out_d[:])

                mean = wp.tile([O, 1], f32)
                nc.vector.tensor_scalar(mean[:], gst[:, 0:1], 1.0 / cnt, None,
                                        ALU.mult)
                ex2 = wp.tile([O, 1], f32)
                nc.vector.tensor_scalar(ex2[:], gst[:, 1:2], 1.0 / cnt, None,
                                        ALU.mult)
                var = wp.tile([O, 1], f32)
                nc.vector.tensor_tensor(var[:], mean[:], mean[:], ALU.mult)
                nc.vector.tensor_sub(var[:], ex2[:], var[:])
                epst = wp.tile([O, 1], f32)
                nc.vector.memset(epst[:], EPS)
                std = wp.tile([O, 1], f32)
                nc.scalar.activation(std[:], var[:], AF.Sqrt, bias=epst[:])
                inv = wp.tile([O, 1], f32)
                nc.vector.reciprocal(inv[:], std[:])
                nbias = wp.tile([O, 1], f32)
                nc.vector.tensor_tensor(nbias[:], mean[:], inv[:], ALU.mult)
                nc.vector.tensor_scalar(nbias[:], nbias[:], -1.0, None,
                                        ALU.mult)

                sc_bc = wp.tile([128, 2], f32)   # col0=scale col1=bias, 2 slots
                for slot in range(2):
                    nc.gpsimd.dma_start(out=sc_bc[slot * O:(slot + 1) * O, 0:1],
                                        in_=inv[:])
                    nc.gpsimd.dma_start(out=sc_bc[slot * O:(slot + 1) * O, 1:2],
                                        in_=nbias[:])

                # ---- normalize + writeout ----
                for pair in range(NPAIRS):
                    sl = out_sb[:, pair, :, :]
                    if pair % 2 == 0:
                        nc.vector.tensor_scalar(sl, sl, sc_bc[:, 0:1],
                                                sc_bc[:, 1:2], ALU.mult,
                                                ALU.add)
                    else:
                        nc.scalar.activation(sl, sl, AF.Identity,
                                             bias=sc_bc[:, 1:2],
                                             scale=sc_bc[:, 0:1])
                    for slot in range(2):
                        b = slot * NPAIRS + pair
                        nc.gpsimd.dma_start(
                            out=out_ext[b],
                            in_=out_sb[slot * O:(slot + 1) * O, pair, :, :])

    nc.compile()
    return nc


def kernel(**inputs):
    from concourse.bass_utils import run_bass_kernel_spmd

    key = NCORES
    if key not in _CACHE:
        _CACHE[key] = _build(NCORES)
    nc = _CACHE[key]

    x = np.ascontiguousarray(inputs["x"], dtype=np.float32)
    shared = {
        "cheb": np.ascontiguousarray(inputs["cheb"], dtype=np.float32),
        "nodevec1": np.ascontiguousarray(inputs["nodevec1"], dtype=np.float32),
        "nodevec2": np.ascontiguousarray(inputs["nodevec2"], dtype=np.float32),
        "alpha": np.ascontiguousarray(inputs["alpha"], dtype=np.float32),
        "Theta": np.ascontiguousarray(inputs["Theta"], dtype=np.float32),
    }
    in_maps = [dict(shared, x=x[i * BLOC:(i + 1) * BLOC]) for i in range(NCORES)]
    res = run_bass_kernel_spmd(nc, in_maps, list(range(NCORES)))
    return np.concatenate([res.results[i]["out"] for i in range(NCORES)], axis=0)
